# revision 1
# baseline (speedup 1.0000x reference)
"""Bass/Trainium2 kernel for nn_Greedy_GLM (GLM with synaptic filtering + dendritic tree).

Strategy: data-parallel over the time axis T=20000 across 8 NeuronCores
(2500 steps each, with a 256-step halo re-computed locally for the causal
length-201 synaptic filter). Per core:

  1. Projection  syn_e = S_e @ C_syn_e.T  via TensorE: S_e is passed from the
     host pre-transposed ([E, T] layout) and cast to bf16 (exact: spikes are
     0/1).  C is split into bf16 hi+lo parts so the f32 product is recovered
     to ~2^-16.  Contraction over E runs on the partition axis.
  2. Depthwise causal conv (kernel length 201) as 3 Toeplitz [128,128]
     matmuls per channel in f32 (out chunk n = sum_m K_m @ in_{n+2-m}).
  3. Binary-tree recurrence over 20 subunits, level-batched on a
     [128 part = t%128, 20ch x 20chunk] tile with strided APs.

The Bass program is input-independent (all parameter data arrives as device
inputs), so it is built/compiled once and cached for repeat calls.
"""

import numpy as np
import ml_dtypes

import concourse.bacc as bacc
import concourse.tile as tile
import concourse.mybir as mybir
from concourse.bass_utils import run_bass_kernel_spmd

BF16 = ml_dtypes.bfloat16

SUB = 20          # subunits / channels
T_NO = 201        # synaptic kernel length
E_NO, I_NO = 2000, 500
T_DATA = 20000
N_CORES = 8
TC = T_DATA // N_CORES      # 2500 timesteps per core
NKO = 20                    # output chunks of 128 (2560 >= 2500)
NKI = 22                    # input chunks of 128 (incl. 2-chunk left halo)
TIN = NKI * 128             # 2816 local input timesteps (t_local = -256..2559)
EP, IP = 2048, 512          # E/I padded to multiples of 128
SE_SUB, SI_SUB = EP // 128, IP // 128

TRACE = False               # test.py sets True to capture a profile
LAST_RESULT = None

_CACHED_NC = None


def _build_nc():
    f32, bf = mybir.dt.float32, mybir.dt.bfloat16
    nc = bacc.Bacc(None)
    se = nc.dram_tensor("se", [EP, TIN], bf, kind="ExternalInput")
    si = nc.dram_tensor("si", [IP, TIN], bf, kind="ExternalInput")
    ce = nc.dram_tensor("ce", [EP, 40], bf, kind="ExternalInput")   # cols 0:20 hi, 20:40 lo
    ci = nc.dram_tensor("ci", [IP, 40], bf, kind="ExternalInput")
    # Toeplitz conv weights, pre-swizzled to [p, (group, i)]; group = ei*60? no:
    # e channels: group g = j*3 + m  (cols g*128..), i channels: g = 60 + j*3 + m
    tz = nc.dram_tensor("tz", [128, 120 * 128], f32, kind="ExternalInput")
    # misc broadcast rows: cols 0:400 theta (ch-major), 400:800 gain, 800:820 V_o
    misc = nc.dram_tensor("misc", [128, 840], f32, kind="ExternalInput")
    vout = nc.dram_tensor("vout", [128, NKO], f32, kind="ExternalOutput")

    se_v = se.rearrange("(s p) t -> p s t", p=128)
    si_v = si.rearrange("(s p) t -> p s t", p=128)

    with tile.TileContext(nc) as tc:
        with (
            tc.tile_pool(name="const", bufs=1) as cpool,
            tc.tile_pool(name="work", bufs=3) as wpool,
            tc.tile_pool(name="ps", bufs=2, space="PSUM") as ppool,
        ):
            se_t = cpool.tile([128, SE_SUB * TIN], bf)
            si_t = cpool.tile([128, SI_SUB * TIN], bf)
            se_tv = se_t.rearrange("p (s t) -> p s t", s=SE_SUB)
            si_tv = si_t.rearrange("p (s t) -> p s t", s=SI_SUB)
            ce_t = cpool.tile([128, SE_SUB * 40], bf)
            ci_t = cpool.tile([128, SI_SUB * 40], bf)
            misc_t = cpool.tile([128, 840], f32)
            syn_e = cpool.tile([128, SUB * NKI], f32)   # col = ch*22 + in_chunk
            syn_i = cpool.tile([128, SUB * NKI], f32)
            recur = cpool.tile([128, SUB * NKO], f32)   # col = ch*20 + out_chunk
            sub = cpool.tile([128, SUB * NKO], f32)
            subg = cpool.tile([128, SUB * NKO], f32)

            nc.sync.dma_start(ce_t.rearrange("p (s n) -> p s n", n=40),
                              ce.rearrange("(s p) n -> p s n", p=128))
            nc.sync.dma_start(ci_t.rearrange("p (s n) -> p s n", n=40),
                              ci.rearrange("(s p) n -> p s n", p=128))
            nc.sync.dma_start(misc_t[:], misc[:])

            syn_e_v = syn_e.rearrange("p (c k) -> p c k", c=SUB)
            syn_i_v = syn_i.rearrange("p (c k) -> p c k", c=SUB)

            # ---- projection: syn[t, ch] over 22 input chunks ----
            for g in range(NKI // 2):  # paired-chunk DMA groups (1 MB each)
                sl = slice(g * 256, (g + 1) * 256)
                nc.sync.dma_start(se_tv[:, :, sl], se_v[:, :, sl])
                nc.sync.dma_start(si_tv[:, :, sl], si_v[:, :, sl])

            for k in range(NKI):
                pe_ = ppool.tile([128, SUB], f32, name=f"pe{k}", tag="pe")
                for s in range(SE_SUB):
                    w = se_t[:, s * TIN + k * 128: s * TIN + (k + 1) * 128]
                    nc.tensor.matmul(pe_[:], w, ce_t[:, s * 40: s * 40 + 20],
                                     start=(s == 0), stop=False)
                    nc.tensor.matmul(pe_[:], w, ce_t[:, s * 40 + 20: s * 40 + 40],
                                     start=False, stop=(s == SE_SUB - 1))
                nc.vector.tensor_copy(syn_e_v[:, :, k], pe_[:])

                pi_ = ppool.tile([128, SUB], f32, name=f"pi{k}", tag="pi")
                for s in range(SI_SUB):
                    w = si_t[:, s * TIN + k * 128: s * TIN + (k + 1) * 128]
                    nc.tensor.matmul(pi_[:], w, ci_t[:, s * 40: s * 40 + 20],
                                     start=(s == 0), stop=False)
                    nc.tensor.matmul(pi_[:], w, ci_t[:, s * 40 + 20: s * 40 + 40],
                                     start=False, stop=(s == SI_SUB - 1))
                nc.vector.tensor_copy(syn_i_v[:, :, k], pi_[:])

            # ---- depthwise causal conv: out chunk n = sum_m K_m @ in_{n+2-m} ----
            for j in range(SUB):
                tzj = wpool.tile([128, 768], f32, name=f"tz{j}", tag="tzj")
                nc.sync.dma_start(tzj[:, 0:384], tz[:, j * 384: (j + 1) * 384])
                nc.sync.dma_start(tzj[:, 384:768],
                                  tz[:, 7680 + j * 384: 7680 + (j + 1) * 384])
                pc = ppool.tile([128, NKO], f32, name=f"pc{j}", tag="pc")
                for m in range(3):
                    nc.tensor.matmul(pc[:], tzj[:, m * 128: (m + 1) * 128],
                                     syn_e[:, j * NKI + 2 - m: j * NKI + 22 - m],
                                     start=(m == 0), stop=False)
                for m in range(3):
                    nc.tensor.matmul(pc[:], tzj[:, 384 + m * 128: 384 + (m + 1) * 128],
                                     syn_i[:, j * NKI + 2 - m: j * NKI + 22 - m],
                                     start=False, stop=(m == 2))
                # + Theta[j]
                nc.vector.tensor_add(recur[:, j * NKO: (j + 1) * NKO], pc[:],
                                     misc_t[:, j * NKO: (j + 1) * NKO])

            # ---- tree recurrence, level-batched ----
            TANH = mybir.ActivationFunctionType.Tanh
            rv = recur.rearrange("p (c k) -> p c k", c=SUB)
            gv = subg.rearrange("p (c k) -> p c k", c=SUB)

            def gain_cols(a, b):           # gain block for channels [a, b)
                return misc_t[:, 400 + a * NKO: 400 + b * NKO]

            # leaves: nodes 10..19
            nc.scalar.activation(sub[:, 200:400], recur[:, 200:400], TANH)
            nc.vector.tensor_mul(subg[:, 200:400], sub[:, 200:400], gain_cols(10, 20))
            # nodes 7,8,9 (children 15/16, 17/18, 19)
            t3 = wpool.tile([128, 60], f32, tag="t3", bufs=1)
            t3v = t3.rearrange("p (c k) -> p c k", c=3)
            nc.vector.tensor_add(t3v[:, 0:2, :], gv[:, 15:19:2, :], gv[:, 16:20:2, :])
            nc.vector.tensor_copy(t3v[:, 2:3, :], gv[:, 19:20, :])
            nc.vector.tensor_add(t3[:], t3[:], recur[:, 140:200])
            nc.scalar.activation(sub[:, 140:200], t3[:], TANH)
            nc.vector.tensor_mul(subg[:, 140:200], sub[:, 140:200], gain_cols(7, 10))
            # nodes 3..6 (children 7..14)
            t4 = wpool.tile([128, 80], f32, tag="t4", bufs=1)
            t4v = t4.rearrange("p (c k) -> p c k", c=4)
            nc.vector.tensor_add(t4v[:], gv[:, 7:14:2, :], gv[:, 8:15:2, :])
            nc.vector.tensor_add(t4[:], t4[:], recur[:, 60:140])
            nc.scalar.activation(sub[:, 60:140], t4[:], TANH)
            nc.vector.tensor_mul(subg[:, 60:140], sub[:, 60:140], gain_cols(3, 7))
            # nodes 1,2 (children 3..6)
            t2 = wpool.tile([128, 40], f32, tag="t2", bufs=1)
            t2v = t2.rearrange("p (c k) -> p c k", c=2)
            nc.vector.tensor_add(t2v[:], gv[:, 3:6:2, :], gv[:, 4:7:2, :])
            nc.vector.tensor_add(t2[:], t2[:], recur[:, 20:60])
            nc.scalar.activation(sub[:, 20:60], t2[:], TANH)
            nc.vector.tensor_mul(subg[:, 20:60], sub[:, 20:60], gain_cols(1, 3))
            # root: node 0 (children 1, 2)
            t1 = wpool.tile([128, 20], f32, tag="t1", bufs=1)
            nc.vector.tensor_add(t1[:], gv[:, 1:2, :], gv[:, 2:3, :])
            nc.vector.tensor_add(t1[:], t1[:], recur[:, 0:20])
            nc.scalar.activation(sub[:, 0:20], t1[:], TANH)
            # V = sub[:,0]*gain[0] + V_o
            vt = wpool.tile([128, NKO], f32, tag="vt", bufs=1)
            nc.vector.tensor_mul(vt[:], sub[:, 0:20], gain_cols(0, 1))
            nc.vector.tensor_add(vt[:], vt[:], misc_t[:, 800:820])
            nc.sync.dma_start(vout[:], vt[:])

    nc.finalize()
    return nc


def _softmax0(x):
    m = x.max(axis=0, keepdims=True)
    e = np.exp(x - m)
    return e / e.sum(axis=0, keepdims=True)


def _host_filters(W_syn, Tau_syn, Delta_syn):
    t = np.arange(T_NO, dtype=np.float32)[None, :]
    t_e = np.maximum(t - np.exp(Delta_syn[:, 0:1]), 0.0)
    t_i = np.maximum(t - np.exp(Delta_syn[:, 1:2]), 0.0)
    tt_e = t_e / np.exp(Tau_syn[:, 0:1])
    tt_i = t_i / np.exp(Tau_syn[:, 1:2])
    e_kern = tt_e * np.exp(-tt_e) * np.exp(W_syn[:, 0:1])
    i_kern = tt_i * np.exp(-tt_i) * (-np.exp(W_syn[:, 1:2]))
    return e_kern.astype(np.float32), i_kern.astype(np.float32)


def _toeplitz(kern):
    """kern [20, 201] -> K[j, m, p, i] = kern[j, 128m + i - p] (0 outside)."""
    i = np.arange(128)[None, :]
    p = np.arange(128)[:, None]
    out = np.zeros((SUB, 3, 128, 128), np.float32)
    for m in range(3):
        idx = 128 * m + i - p
        valid = (idx >= 0) & (idx < T_NO)
        out[:, m] = np.where(valid[None], kern[:, np.clip(idx, 0, T_NO - 1)], 0.0)
    return out


def _hi_lo(C, rows):
    """C [sub, n] f32 -> [rows, 40] bf16 (C.T padded; cols 0:20 hi, 20:40 lo)."""
    Ct = np.zeros((rows, SUB), np.float32)
    Ct[: C.shape[1]] = C.T
    hi = Ct.astype(BF16)
    lo = (Ct - hi.astype(np.float32)).astype(BF16)
    return np.ascontiguousarray(np.concatenate([hi, lo], axis=1))


def kernel(S_e, S_i, C_den, W_syn, Tau_syn, Delta_syn, W_sub, V_o, Theta,
           C_syn_e_raw, C_syn_i_raw, **_ignored):
    global _CACHED_NC, LAST_RESULT
    f = np.float32
    S_e, S_i = np.asarray(S_e, f), np.asarray(S_i, f)
    W_syn, Tau_syn, Delta_syn = (np.asarray(x, f) for x in (W_syn, Tau_syn, Delta_syn))
    W_sub, V_o, Theta = np.asarray(W_sub, f), np.asarray(V_o, f), np.asarray(Theta, f)

    # ---- host-side small math (same ops as the reference) ----
    C_syn_e = _softmax0(np.asarray(C_syn_e_raw, f))
    C_syn_i = _softmax0(np.asarray(C_syn_i_raw, f))
    e_kern, i_kern = _host_filters(W_syn, Tau_syn, Delta_syn)
    out_filters = np.vstack([e_kern, i_kern])
    gain = np.exp(W_sub)

    # Toeplitz conv weights -> [p, (group, i)] with e groups first
    TZ = np.concatenate([_toeplitz(e_kern), _toeplitz(i_kern)], axis=0)  # [40,3,128,128]
    TZ = TZ.reshape(120, 128, 128).transpose(1, 0, 2).reshape(128, 120 * 128)
    TZ = np.ascontiguousarray(TZ)

    ce_host = _hi_lo(C_syn_e, EP)
    ci_host = _hi_lo(C_syn_i, IP)

    misc = np.zeros((1, 840), np.float32)
    misc[0, 0:400] = np.repeat(Theta, NKO)
    misc[0, 400:800] = np.repeat(gain, NKO)
    misc[0, 800:820] = V_o[0]
    misc = np.ascontiguousarray(np.broadcast_to(misc, (128, 840)))

    # spikes: transpose, pad (256 halo left, tail right), cast to bf16 (exact)
    pad_cols = 256 + T_DATA + (NKO * 128 - TC)   # 20316
    SeT = np.zeros((EP, pad_cols), BF16)
    SeT[:E_NO, 256: 256 + T_DATA] = S_e.T
    SiT = np.zeros((IP, pad_cols), BF16)
    SiT[:I_NO, 256: 256 + T_DATA] = S_i.T

    in_maps = []
    for c in range(N_CORES):
        off = c * TC
        in_maps.append({
            "se": np.ascontiguousarray(SeT[:, off: off + TIN]),
            "si": np.ascontiguousarray(SiT[:, off: off + TIN]),
            "ce": ce_host, "ci": ci_host, "tz": TZ, "misc": misc,
        })

    if _CACHED_NC is None:
        _CACHED_NC = _build_nc()

    res = run_bass_kernel_spmd(_CACHED_NC, in_maps, core_ids=list(range(N_CORES)),
                               trace=TRACE)
    LAST_RESULT = res

    V = np.empty(T_DATA, np.float32)
    for c in range(N_CORES):
        V[c * TC: (c + 1) * TC] = res.results[c]["vout"].T.reshape(-1)[:TC]

    return V, out_filters, C_syn_e, C_syn_i
